# revision 21
# baseline (speedup 1.0000x reference)
"""BalanceMSELoss on 8 Trainium2 NeuronCores.

reference:
    inside = box mask from boxes (per batch), d2 = (input-target)^2
    loss = 0.5 * sum(d2*m)/sum(m) + 0.5 * sum(d2*(1-m))/sum(1-m)
    (the reference mask is (B,1,H,W): its count excludes the C factor
     while its masked sum spans all C channels)

Sharding: batch dim B=32 -> 8 cores x 4 images (data parallel).

Per core, per (b, c) slab [512, 512] viewed as [128p, 4t, 512w]:
  - DVE:  diff = input - target            (fp32 in, fp16 out)
  - ACT:  d2 = Square(diff)                (fp16)
  - PE:   per t, matmul lhsT=[ymask_col, ones_col] ([128,2] fp16) x d2
          -> PSUM[2, 512] accumulated over (c, t): row 0 = y-masked
          column sums, row 1 = unmasked column sums.
The y box mask rides in as *data* (per-core input), so the single SPMD
program is identical on every core. The x-direction mask is applied on
host: inside_b = dot(psum_row0, xmask_b); total_b = sum(psum_row1).
Host reduces in float64, computes counts from boxes, forms the loss.
"""

import numpy as np

N_CORES = 8
B, C, H, W = 32, 3, 512, 512
BL = B // N_CORES  # batches per core
P = 128
T = H // P  # h-tiles per image
ETA = 1.0

_CACHE = {}


def _build_program():
    import concourse.bacc as bacc
    import concourse.mybir as mybir
    import concourse.tile as tile

    nc = bacc.Bacc("TRN2", debug=False, target_bir_lowering=False,
                   num_devices=N_CORES)
    f32 = mybir.dt.float32
    f16 = mybir.dt.float16
    inp = nc.dram_tensor("inp", [BL, C, H, W], f32, kind="ExternalInput")
    tgt = nc.dram_tensor("tgt", [BL, C, H, W], f32, kind="ExternalInput")
    # [P, BL*T, 2] fp16: [...,0] = y box mask for (b, t), [...,1] = 1.0
    ym2 = nc.dram_tensor("ym2", [P, BL * T, 2], f16, kind="ExternalInput")
    # [BL, 2, 512] f32: [b, 0, w] = sum_{c,t,h} ymask*d2; [b, 1, w] = sum d2
    ycols = nc.dram_tensor("ycols", [BL, 2, W], f32, kind="ExternalOutput")

    inp_ap = inp.ap()
    tgt_ap = tgt.ap()
    ycols_ap = ycols.ap()
    Square = mybir.ActivationFunctionType.Square

    with tile.TileContext(nc) as tc:
        with tc.tile_pool(name="singles", bufs=1) as singles, \
             tc.tile_pool(name="io", bufs=7) as io_pool, \
             tc.tile_pool(name="work", bufs=3) as work, \
             tc.tile_pool(name="psum", bufs=1, space="PSUM") as psum_pool:
            ym = singles.tile([P, BL * T, 2], f16)
            nc.sync.dma_start(out=ym, in_=ym2.ap())

            for b in range(BL):
                ps = psum_pool.tile([2, W], f32, tag=f"ps{b}")
                for c in range(C):
                    it = io_pool.tile([P, T, W], f32, tag="it")
                    tt = io_pool.tile([P, T, W], f32, tag="tt")
                    # issue the two loads from different engines so the
                    # descriptor generation runs on two HWDGE queues
                    nc.sync.dma_start(
                        out=it,
                        in_=inp_ap[b, c].rearrange("(t p) w -> p t w", p=P))
                    nc.scalar.dma_start(
                        out=tt,
                        in_=tgt_ap[b, c].rearrange("(t p) w -> p t w", p=P))

                    diff = work.tile([P, T, W], f16, tag="diff")
                    nc.vector.tensor_sub(diff, it, tt)

                    d2 = work.tile([P, T, W], f16, tag="d2")
                    nc.scalar.activation(out=d2, in_=diff, func=Square)

                    for t in range(T):
                        nc.tensor.matmul(
                            ps[:], ym[:, b * T + t, :], d2[:, t, :],
                            start=(c == 0 and t == 0),
                            stop=(c == C - 1 and t == T - 1))

                # PSUM can't be a DMA source; bounce through SBUF
                sb = work.tile([2, W], f32, tag="psout")
                nc.vector.tensor_copy(sb, ps[:])
                nc.sync.dma_start(out=ycols_ap[b], in_=sb[:])

    nc.compile()
    return nc


def _get_exec():
    """Build program once and wrap it in a cached jitted shard_map callable
    (mirrors bass2jax.run_bass_via_pjrt's multi-core branch, but reusable
    across calls so we don't re-trace/re-compile per invocation)."""
    if "exec" in _CACHE:
        return _CACHE["exec"]
    import jax
    from jax.experimental.shard_map import shard_map
    from jax.sharding import Mesh, PartitionSpec

    import concourse.mybir as mybir
    from concourse import bass2jax

    nc = _build_program()
    bass2jax.install_neuronx_cc_hook()

    partition_name = nc.partition_id_tensor.name if nc.partition_id_tensor else None
    in_names, out_names, out_avals = [], [], []
    for alloc in nc.m.functions[0].allocations:
        if not isinstance(alloc, mybir.MemoryLocationSet):
            continue
        name = alloc.memorylocations[0].name
        if alloc.kind == "ExternalInput":
            if name != partition_name:
                in_names.append(name)
        elif alloc.kind == "ExternalOutput":
            out_avals.append(jax.core.ShapedArray(
                tuple(alloc.tensor_shape), mybir.dt.np(alloc.dtype)))
            out_names.append(name)
    n_params = len(in_names)
    n_outs = len(out_names)
    all_in_names = in_names + out_names + (
        [partition_name] if partition_name else [])
    donate = tuple(range(n_params, n_params + n_outs))

    def _body(*args):
        operands = list(args)
        if partition_name is not None:
            operands.append(bass2jax.partition_id_tensor())
        return tuple(bass2jax._bass_exec_p.bind(
            *operands,
            out_avals=tuple(out_avals),
            in_names=tuple(all_in_names),
            out_names=tuple(out_names),
            lowering_input_output_aliases=(),
            sim_require_finite=True,
            sim_require_nnan=True,
            nc=nc,
        ))

    devices = jax.devices()[:N_CORES]
    assert len(devices) == N_CORES
    mesh = Mesh(np.asarray(devices), ("core",))
    in_specs = (PartitionSpec("core"),) * (n_params + n_outs)
    out_specs = (PartitionSpec("core"),) * n_outs
    sharded = jax.jit(
        shard_map(_body, mesh=mesh, in_specs=in_specs, out_specs=out_specs,
                  check_rep=False),
        donate_argnums=donate, keep_unused=True)

    ex = dict(nc=nc, sharded=sharded, in_names=in_names, out_names=out_names,
              out_avals=out_avals, mesh=mesh, n_params=n_params)
    _CACHE["exec"] = ex
    return ex


def _prepare(input, target, boxes):
    """Host-side prep: box coords, masks, concatenated inputs, counts."""
    input = np.asarray(input, dtype=np.float32)
    target = np.asarray(target, dtype=np.float32)
    boxes = np.asarray(boxes, dtype=np.float32)

    # Box coordinates exactly as the reference computes them (f32 multiply,
    # floor, int cast).
    x1 = np.floor(boxes[:, 0] * np.float32(W)).astype(np.int64)
    y1 = np.floor(boxes[:, 1] * np.float32(H)).astype(np.int64)
    bw = np.floor(boxes[:, 2] * np.float32(W)).astype(np.int64)
    bh = np.floor(boxes[:, 3] * np.float32(H)).astype(np.int64)

    xs = np.arange(W)
    ys = np.arange(H)
    # [B, W] / [B, H] 0/1 indicators of the box interval (inclusive ends)
    xmask_full = ((xs[None, :] >= x1[:, None]) &
                  (xs[None, :] <= (x1 + bw)[:, None])).astype(np.float64)
    ymask_full = ((ys[None, :] >= y1[:, None]) &
                  (ys[None, :] <= (y1 + bh)[:, None])).astype(np.float64)

    # ym2 global layout [N_CORES*P, BL*T, 2]: core k rows k*P..(k+1)*P;
    # column (b*T + t) holds image (k*BL+b) rows t*128..t*128+127; last dim
    # is (ymask, ones).
    ym = ymask_full.reshape(N_CORES, BL, T, P).transpose(0, 3, 1, 2)  # k,p,b,t
    ym2 = np.empty((N_CORES, P, BL * T, 2), dtype=np.float16)
    ym2[..., 0] = ym.reshape(N_CORES, P, BL * T).astype(np.float16)
    ym2[..., 1] = np.float16(1.0)

    concat = {
        "inp": input,   # [B, C, H, W] -> per-core [BL, C, H, W]
        "tgt": target,
        "ym2": np.ascontiguousarray(ym2.reshape(N_CORES * P, BL * T, 2)),
    }

    # NB: reference mask is (B,1,H,W) — counts exclude the C factor.
    ins_cnt = float((xmask_full.sum(axis=1) * ymask_full.sum(axis=1)).sum())
    tot_cnt = float(B * H * W)
    return concat, ins_cnt, tot_cnt, xmask_full


def _run(ex, concat):
    import jax
    concat_in = [concat[name] for name in ex["in_names"]]
    zeros = [np.zeros((N_CORES * av.shape[0], *av.shape[1:]), av.dtype)
             for av in ex["out_avals"]]
    out_arrs = ex["sharded"](*concat_in, *zeros)
    out_arrs = jax.block_until_ready(out_arrs)
    return {name: np.asarray(out_arrs[i])
            for i, name in enumerate(ex["out_names"])}


def kernel(input, target, boxes):
    ex = _get_exec()
    concat, ins_cnt, tot_cnt, xmask_full = _prepare(input, target, boxes)
    outs = _run(ex, concat)

    ycols = outs["ycols"].astype(np.float64).reshape(B, 2, W)
    ins_sum = float((ycols[:, 0, :] * xmask_full).sum())
    tot_sum = float(ycols[:, 1, :].sum())

    inside_loss = ins_sum / ins_cnt
    outside_loss = (tot_sum - ins_sum) / (tot_cnt - ins_cnt)
    loss = (0.5 * inside_loss + 0.5 * outside_loss) * ETA
    return np.asarray(loss, dtype=np.float32)


# revision 22
# speedup vs baseline: 1.1632x; 1.1632x over previous
"""BalanceMSELoss on 8 Trainium2 NeuronCores.

reference:
    inside = box mask from boxes (per batch), d2 = (input-target)^2
    loss = 0.5 * sum(d2*m)/sum(m) + 0.5 * sum(d2*(1-m))/sum(1-m)
    (the reference mask is (B,1,H,W): its count excludes the C factor
     while its masked sum spans all C channels)

Sharding: batch dim B=32 -> 8 cores x 4 images (data parallel).

Per core, per (b, c) slab [512, 512] viewed as [128p, 4t, 512w]:
  - DVE:  diff = input - target            (fp32 in, fp16 out)
  - ACT:  d2 = Square(diff)                (fp16)
  - PE:   per t, matmul lhsT=[ymask_col, ones_col] ([128,2] fp16) x d2
          -> PSUM[2, 512] accumulated over (c, t): row 0 = y-masked
          column sums, row 1 = unmasked column sums.
The y box mask rides in as *data* (per-core input), so the single SPMD
program is identical on every core. The x-direction mask is applied on
host: inside_b = dot(psum_row0, xmask_b); total_b = sum(psum_row1).
Host reduces in float64, computes counts from boxes, forms the loss.
"""

import numpy as np

N_CORES = 8
B, C, H, W = 32, 3, 512, 512
BL = B // N_CORES  # batches per core
P = 128
T = H // P  # h-tiles per image
ETA = 1.0

_CACHE = {}


def _build_program():
    import concourse.bacc as bacc
    import concourse.mybir as mybir
    import concourse.tile as tile

    nc = bacc.Bacc("TRN2", debug=False, target_bir_lowering=False,
                   num_devices=N_CORES)
    f32 = mybir.dt.float32
    f16 = mybir.dt.float16
    inp = nc.dram_tensor("inp", [BL, C, H, W], f32, kind="ExternalInput")
    tgt = nc.dram_tensor("tgt", [BL, C, H, W], f32, kind="ExternalInput")
    # [P, BL*T, 2] fp16: [...,0] = y box mask for (b, t), [...,1] = 1.0
    ym2 = nc.dram_tensor("ym2", [P, BL * T, 2], f16, kind="ExternalInput")
    # [BL, 2, 512] f32: [b, 0, w] = sum_{c,t,h} ymask*d2; [b, 1, w] = sum d2
    ycols = nc.dram_tensor("ycols", [BL, 2, W], f32, kind="ExternalOutput")

    inp_ap = inp.ap()
    tgt_ap = tgt.ap()
    ycols_ap = ycols.ap()
    Square = mybir.ActivationFunctionType.Square

    with tile.TileContext(nc) as tc:
        with tc.tile_pool(name="singles", bufs=1) as singles, \
             tc.tile_pool(name="io", bufs=7) as io_pool, \
             tc.tile_pool(name="work", bufs=3) as work, \
             tc.tile_pool(name="psum", bufs=1, space="PSUM") as psum_pool:
            ym = singles.tile([P, BL * T, 2], f16)
            nc.sync.dma_start(out=ym, in_=ym2.ap())

            for b in range(BL):
                ps = psum_pool.tile([2, W], f32, tag=f"ps{b}")
                for c in range(C):
                    it = io_pool.tile([P, T, W], f32, tag="it")
                    tt = io_pool.tile([P, T, W], f32, tag="tt")
                    # issue the two loads from different engines so the
                    # descriptor generation runs on two HWDGE queues
                    nc.sync.dma_start(
                        out=it,
                        in_=inp_ap[b, c].rearrange("(t p) w -> p t w", p=P))
                    nc.scalar.dma_start(
                        out=tt,
                        in_=tgt_ap[b, c].rearrange("(t p) w -> p t w", p=P))

                    diff = work.tile([P, T, W], f16, tag="diff")
                    nc.vector.tensor_sub(diff, it, tt)

                    d2 = work.tile([P, T, W], f16, tag="d2")
                    nc.vector.tensor_mul(d2, diff, diff)

                    for t in range(T):
                        nc.tensor.matmul(
                            ps[:], ym[:, b * T + t, :], d2[:, t, :],
                            start=(c == 0 and t == 0),
                            stop=(c == C - 1 and t == T - 1))

                # PSUM can't be a DMA source; bounce through SBUF
                sb = work.tile([2, W], f32, tag="psout")
                nc.vector.tensor_copy(sb, ps[:])
                nc.sync.dma_start(out=ycols_ap[b], in_=sb[:])

    nc.compile()
    return nc


def _get_exec():
    """Build program once and wrap it in a cached jitted shard_map callable
    (mirrors bass2jax.run_bass_via_pjrt's multi-core branch, but reusable
    across calls so we don't re-trace/re-compile per invocation)."""
    if "exec" in _CACHE:
        return _CACHE["exec"]
    import jax
    from jax.experimental.shard_map import shard_map
    from jax.sharding import Mesh, PartitionSpec

    import concourse.mybir as mybir
    from concourse import bass2jax

    nc = _build_program()
    bass2jax.install_neuronx_cc_hook()

    partition_name = nc.partition_id_tensor.name if nc.partition_id_tensor else None
    in_names, out_names, out_avals = [], [], []
    for alloc in nc.m.functions[0].allocations:
        if not isinstance(alloc, mybir.MemoryLocationSet):
            continue
        name = alloc.memorylocations[0].name
        if alloc.kind == "ExternalInput":
            if name != partition_name:
                in_names.append(name)
        elif alloc.kind == "ExternalOutput":
            out_avals.append(jax.core.ShapedArray(
                tuple(alloc.tensor_shape), mybir.dt.np(alloc.dtype)))
            out_names.append(name)
    n_params = len(in_names)
    n_outs = len(out_names)
    all_in_names = in_names + out_names + (
        [partition_name] if partition_name else [])
    donate = tuple(range(n_params, n_params + n_outs))

    def _body(*args):
        operands = list(args)
        if partition_name is not None:
            operands.append(bass2jax.partition_id_tensor())
        return tuple(bass2jax._bass_exec_p.bind(
            *operands,
            out_avals=tuple(out_avals),
            in_names=tuple(all_in_names),
            out_names=tuple(out_names),
            lowering_input_output_aliases=(),
            sim_require_finite=True,
            sim_require_nnan=True,
            nc=nc,
        ))

    devices = jax.devices()[:N_CORES]
    assert len(devices) == N_CORES
    mesh = Mesh(np.asarray(devices), ("core",))
    in_specs = (PartitionSpec("core"),) * (n_params + n_outs)
    out_specs = (PartitionSpec("core"),) * n_outs
    sharded = jax.jit(
        shard_map(_body, mesh=mesh, in_specs=in_specs, out_specs=out_specs,
                  check_rep=False),
        donate_argnums=donate, keep_unused=True)

    ex = dict(nc=nc, sharded=sharded, in_names=in_names, out_names=out_names,
              out_avals=out_avals, mesh=mesh, n_params=n_params)
    _CACHE["exec"] = ex
    return ex


def _prepare(input, target, boxes):
    """Host-side prep: box coords, masks, concatenated inputs, counts."""
    input = np.asarray(input, dtype=np.float32)
    target = np.asarray(target, dtype=np.float32)
    boxes = np.asarray(boxes, dtype=np.float32)

    # Box coordinates exactly as the reference computes them (f32 multiply,
    # floor, int cast).
    x1 = np.floor(boxes[:, 0] * np.float32(W)).astype(np.int64)
    y1 = np.floor(boxes[:, 1] * np.float32(H)).astype(np.int64)
    bw = np.floor(boxes[:, 2] * np.float32(W)).astype(np.int64)
    bh = np.floor(boxes[:, 3] * np.float32(H)).astype(np.int64)

    xs = np.arange(W)
    ys = np.arange(H)
    # [B, W] / [B, H] 0/1 indicators of the box interval (inclusive ends)
    xmask_full = ((xs[None, :] >= x1[:, None]) &
                  (xs[None, :] <= (x1 + bw)[:, None])).astype(np.float64)
    ymask_full = ((ys[None, :] >= y1[:, None]) &
                  (ys[None, :] <= (y1 + bh)[:, None])).astype(np.float64)

    # ym2 global layout [N_CORES*P, BL*T, 2]: core k rows k*P..(k+1)*P;
    # column (b*T + t) holds image (k*BL+b) rows t*128..t*128+127; last dim
    # is (ymask, ones).
    ym = ymask_full.reshape(N_CORES, BL, T, P).transpose(0, 3, 1, 2)  # k,p,b,t
    ym2 = np.empty((N_CORES, P, BL * T, 2), dtype=np.float16)
    ym2[..., 0] = ym.reshape(N_CORES, P, BL * T).astype(np.float16)
    ym2[..., 1] = np.float16(1.0)

    concat = {
        "inp": input,   # [B, C, H, W] -> per-core [BL, C, H, W]
        "tgt": target,
        "ym2": np.ascontiguousarray(ym2.reshape(N_CORES * P, BL * T, 2)),
    }

    # NB: reference mask is (B,1,H,W) — counts exclude the C factor.
    ins_cnt = float((xmask_full.sum(axis=1) * ymask_full.sum(axis=1)).sum())
    tot_cnt = float(B * H * W)
    return concat, ins_cnt, tot_cnt, xmask_full


def _run(ex, concat):
    import jax
    concat_in = [concat[name] for name in ex["in_names"]]
    zeros = [np.zeros((N_CORES * av.shape[0], *av.shape[1:]), av.dtype)
             for av in ex["out_avals"]]
    out_arrs = ex["sharded"](*concat_in, *zeros)
    out_arrs = jax.block_until_ready(out_arrs)
    return {name: np.asarray(out_arrs[i])
            for i, name in enumerate(ex["out_names"])}


def kernel(input, target, boxes):
    ex = _get_exec()
    concat, ins_cnt, tot_cnt, xmask_full = _prepare(input, target, boxes)
    outs = _run(ex, concat)

    ycols = outs["ycols"].astype(np.float64).reshape(B, 2, W)
    ins_sum = float((ycols[:, 0, :] * xmask_full).sum())
    tot_sum = float(ycols[:, 1, :].sum())

    inside_loss = ins_sum / ins_cnt
    outside_loss = (tot_sum - ins_sum) / (tot_cnt - ins_cnt)
    loss = (0.5 * inside_loss + 0.5 * outside_loss) * ETA
    return np.asarray(loss, dtype=np.float32)
